# revision 1
# baseline (speedup 1.0000x reference)
"""Multi-head causal attention (B=4, S=2048, H=1024, NH=16) on 8 trn2 cores.

Head-sharded tensor parallelism: core i computes heads {2i, 2i+1}.  Each core
runs projections for its 2 heads (fp32r matmuls), causal flash-style attention
in a transposed orientation (scores S^T[k,q] so the P@V contraction needs no
transpose of P), and a partial output projection over its 128 channels.  The
8 partial outputs are summed on the host (the tensor-parallel all-reduce),
plus the output bias.
"""
import numpy as np

import concourse.bacc as bacc
import concourse.tile as tile
from concourse import mybir
from concourse.bass_utils import run_bass_kernel_spmd

F32 = mybir.dt.float32
F32R = mybir.dt.float32r
AF = mybir.ActivationFunctionType

B, S, H, NH = 4, 2048, 1024, 16
HD = H // NH            # 64
NCORES = 8
HPC = NH // NCORES      # 2 heads per core
C = HPC * HD            # 128 channels per core
SCALE = 1.0 / np.sqrt(HD)

QT_W = 256              # q-tile width (columns of S^T tiles)
KC = 128                # k-chunk (contraction tile for P@V)
N_QT = S // QT_W        # 8
N_KC = S // KC          # 16
N_HC = H // 128         # 8 contraction chunks for projections
N_ST = 4                # s-tiles of 512 for projections

_CACHE = {}
PHASES = ("proj", "vtrans", "attn", "oproj")
PROJ_PRIO = 0


def _build_nc():
    nc = bacc.Bacc(name="mha_tp")
    xt_d = nc.dram_tensor("xt", [B, H, S], F32R, kind="ExternalInput")
    wq_d = nc.dram_tensor("wqt", [H, C], F32R, kind="ExternalInput")
    wk_d = nc.dram_tensor("wkt", [H, C], F32R, kind="ExternalInput")
    wv_d = nc.dram_tensor("wvt", [H, C], F32R, kind="ExternalInput")
    wo_d = nc.dram_tensor("wot", [C, H], F32R, kind="ExternalInput")
    bq_d = nc.dram_tensor("bq", [C, 1], F32, kind="ExternalInput")
    bk_d = nc.dram_tensor("bk", [C, 1], F32, kind="ExternalInput")
    bv_d = nc.dram_tensor("bv", [C, 1], F32, kind="ExternalInput")
    mk_d = nc.dram_tensor("maskbuf", [128, 896], F32R, kind="ExternalInput")
    id_d = nc.dram_tensor("ident", [128, 128], F32, kind="ExternalInput")
    on_d = nc.dram_tensor("ones16", [128, N_KC], F32R, kind="ExternalInput")
    out_d = nc.dram_tensor("out", [B, S, H], F32, kind="ExternalOutput")

    with tile.TileContext(nc) as tc:
        with (
            tc.tile_pool(name="const", bufs=1) as cp,
            tc.tile_pool(name="big", bufs=2) as bp,
            tc.tile_pool(name="work", bufs=2) as wp,
            tc.tile_pool(name="xs", bufs=12) as xp,
            tc.tile_pool(name="ps", bufs=1, space="PSUM") as ps,
            tc.tile_pool(name="psmix", bufs=2, space="PSUM") as pm,
        ):
            # ---- constants ----
            wq_s = cp.tile([128, H], F32R)
            wk_s = cp.tile([128, H], F32R)
            wv_s = cp.tile([128, H], F32R)
            wo_s = cp.tile([128, H], F32R)
            mk_s = cp.tile([128, 896], F32R)
            id_s = cp.tile([128, 128], F32)
            on_s = cp.tile([128, N_KC], F32R)
            bq_s = cp.tile([C, 1], F32)
            bk_s = cp.tile([C, 1], F32)
            bv_s = cp.tile([C, 1], F32)
            for w_s, w_d in ((wq_s, wq_d), (wk_s, wk_d), (wv_s, wv_d)):
                nc.scalar.dma_start(
                    w_s.rearrange("p (c d) -> p c d", d=128),
                    w_d.ap().rearrange("(c p) d -> p c d", p=128))
            nc.scalar.dma_start(wo_s[:], wo_d.ap())
            nc.scalar.dma_start(mk_s[:], mk_d.ap())
            nc.scalar.dma_start(id_s[:], id_d.ap())
            nc.scalar.dma_start(on_s[:], on_d.ap())
            nc.scalar.dma_start(bq_s[:], bq_d.ap())
            nc.scalar.dma_start(bk_s[:], bk_d.ap())
            nc.scalar.dma_start(bv_s[:], bv_d.ap())

            tiles = {}

            def emit_proj(b, halves=(0, 1)):
                # ---- projections: QT/KT [128, S] f32r, VT [128, S] f32 ----
                if b not in tiles:
                    qt = bp.tile([128, S], F32R, tag="qt", name=f"qt{b}")
                    kt = bp.tile([128, S], F32R, tag="kt", name=f"kt{b}")
                    vt = bp.tile([128, S], F32, tag="vt", name=f"vt{b}", bufs=1)
                    tiles[b] = {"qt": qt, "kt": kt, "vt": vt}
                qt, kt, vt = tiles[b]["qt"], tiles[b]["kt"], tiles[b]["vt"]
                if True:
                  for half in halves if "proj" in PHASES else []:
                    xts = []
                    for hc in range(N_HC):
                        hsl = slice(hc * 128, (hc + 1) * 128)
                        xt_t = xp.tile([128, 1024], F32R, tag="xt",
                                       name=f"x{b}_{half}_{hc}")
                        nc.sync.dma_start(
                            xt_t[:], xt_d.ap()[b, hsl, half * 1024:(half + 1) * 1024])
                        xts.append(xt_t)
                    for sth in range(2):
                        st = half * 2 + sth
                        ssl = slice(st * 512, (st + 1) * 512)
                        # sequential Q/K/V passes over resident x^T chunks: 2
                        # PSUM slots suffice (pipeline pass i+1 against copy i)
                        for w_s, bias, dst, pnm in ((wq_s, bq_s, qt, "q"),
                                                    (wk_s, bk_s, kt, "k"),
                                                    (wv_s, bv_s, vt, "v")):
                            pp = pm.tile([128, 512], F32, tag="mix",
                                         name=f"pp{pnm}{b}_{st}")
                            for hc in range(N_HC):
                                nc.tensor.matmul(
                                    pp[:], w_s[:, hc * 128:(hc + 1) * 128],
                                    xts[hc][:, sth * 512:(sth + 1) * 512],
                                    start=(hc == 0), stop=(hc == N_HC - 1))
                            nc.vector.tensor_scalar_add(dst[:, ssl], pp[:], bias[:])

            def emit_vtrans(b):
                # ---- V transpose: vn_h [128, 16*65] (ones col at 64 of each 65) ----
                vt = tiles[b]["vt"]
                vna = bp.tile([128, N_KC * (HD + 1)], F32R, tag="vna", name=f"vna{b}")
                vnb = bp.tile([128, N_KC * (HD + 1)], F32R, tag="vnb", name=f"vnb{b}")
                tiles[b]["vna"], tiles[b]["vnb"] = vna, vnb
                for h, vn in ((0, vna), (1, vnb)):
                    vn3 = vn.rearrange("p (c e) -> p c e", e=HD + 1)
                    nc.sync.dma_start(vn3[:, :, HD], on_d.ap())
                for c in range(N_KC) if "vtrans" in PHASES else []:
                    tp = pm.tile([128, 128], F32, tag="mix", name=f"tp{b}_{c}")
                    nc.tensor.transpose(tp[:], vt[:, c * 128:(c + 1) * 128], id_s[:])
                    nc.any.tensor_copy(vna[:, c * (HD + 1): c * (HD + 1) + HD],
                                       tp[:, 0:HD])
                    nc.any.tensor_copy(vnb[:, c * (HD + 1): c * (HD + 1) + HD],
                                       tp[:, HD:2 * HD])

            def emit_attn(b, jlo=0, jhi=N_QT):
                # ---- attention (transposed scores), both heads interleaved ----
                qt, kt = tiles[b]["qt"], tiles[b]["kt"]
                if "ctx" not in tiles[b]:
                    ctx = bp.tile([128, S], F32R, tag="ctx", name=f"ctx{b}")
                    tiles[b]["ctx"] = ctx
                ctx = tiles[b]["ctx"]
                vns = (tiles[b]["vna"], tiles[b]["vnb"])
                for j in range(jlo, jhi) if "attn" in PHASES else []:
                    qsl = slice(j * QT_W, (j + 1) * QT_W)
                    acc = ps.tile([128, 512], F32, tag="acc", name=f"acc{b}_{j}",
                                  bufs=2)
                    nc.vector.memset(acc[:], 0.0)
                    nkc = 2 * (j + 1)              # causal: k-chunks 0..nkc-1
                    n_sc = (nkc + 3) // 4
                    for sc in range(n_sc):
                        cs = [c for c in range(4 * sc, min(4 * sc + 4, nkc))]
                        sts, pts = [], []
                        for h in range(2):
                            st_h = ps.tile([128, 4 * QT_W], F32, tag=f"st{h}",
                                           name=f"st{h}_{b}_{j}_{sc}")
                            pt_h = wp.tile([128, 4 * QT_W], F32R, tag=f"pt{h}",
                                           name=f"pt{h}_{b}_{j}_{sc}", bufs=5)
                            sts.append(st_h)
                            pts.append(pt_h)
                        for c in cs:   # QK: heads adjacent -> row-group concurrency
                            for h in range(2):
                                hsl = slice(h * HD, (h + 1) * HD)
                                nc.tensor.matmul(
                                    sts[h][:, (c - 4 * sc) * QT_W:(c - 4 * sc + 1) * QT_W],
                                    kt[hsl, c * KC:(c + 1) * KC],
                                    qt[hsl, qsl],
                                    start=True, stop=True,
                                )
                        w = len(cs) * QT_W
                        for h in range(2):
                            nc.scalar.activation(pts[h][:, 0:w], sts[h][:, 0:w],
                                                 AF.Exp, scale=float(SCALE))
                        if sc == n_sc - 1:  # diagonal: mask last two k-chunks
                            for h in range(2):
                                for c in (nkc - 2, nkc - 1):
                                    mo = 384 - 128 * (c - 2 * j)  # o = 128*(c-2j)
                                    nc.gpsimd.tensor_mul(
                                        pts[h][:, (c - 4 * sc) * QT_W:(c - 4 * sc + 1) * QT_W],
                                        pts[h][:, (c - 4 * sc) * QT_W:(c - 4 * sc + 1) * QT_W],
                                        mk_s[:, mo:mo + QT_W],
                                    )
                        for c in cs:   # P@V (+ones rowsum row)
                            for h in range(2):
                                nc.tensor.matmul(
                                    acc[0:HD + 1, h * QT_W:(h + 1) * QT_W],
                                    vns[h][:, c * (HD + 1):(c + 1) * (HD + 1)],
                                    pts[h][:, (c - 4 * sc) * QT_W:(c - 4 * sc + 1) * QT_W],
                                    start=False, stop=(c == nkc - 1),
                                    skip_group_check=True,
                                )
                    # normalize: one recip over both heads' rowsum halves,
                    # partition-broadcast on the (idle) gpsimd, one fused mul
                    recip = wp.tile([1, 2 * QT_W], F32, tag="recip",
                                    name=f"rc{b}_{j}")
                    nc.vector.reciprocal(recip[:], acc[HD:HD + 1, :])
                    for h in range(2):
                        asl = slice(h * QT_W, (h + 1) * QT_W)
                        bc_sb = wp.tile([HD, QT_W], F32, tag="bcs",
                                        name=f"bcs{b}_{j}_{h}", bufs=4)
                        nc.gpsimd.partition_broadcast(bc_sb[:], recip[0:1, asl])
                        nc.any.tensor_mul(ctx[h * HD:(h + 1) * HD, qsl],
                                          acc[0:HD, asl], bc_sb[:])

            def emit_oproj(b):
                ctx = tiles[b]["ctx"]
                for qp in range(S // 256) if "oproj" in PHASES else []:
                    osb = wp.tile([128, 2048], F32, tag="osb", name=f"ob{b}_{qp}")
                    for sub in range(2):
                        qc = 2 * qp + sub
                        for half in range(2):
                            osl = slice(half * 512, (half + 1) * 512)
                            op = pm.tile([128, 512], F32, tag="mix",
                                         name=f"op{b}_{qc}_{half}")
                            nc.tensor.matmul(op[:], ctx[:, qc * 128:(qc + 1) * 128],
                                             wo_s[:, osl], start=True, stop=True)
                            nc.vector.tensor_copy(
                                osb[:, sub * 1024 + half * 512:
                                    sub * 1024 + (half + 1) * 512], op[:])
                    nc.sync.dma_start(
                        out_d.ap()[b, qp * 256:(qp + 1) * 256, :]
                        .rearrange("(g q) o -> q g o", g=2),
                        osb.rearrange("p (g o) -> p g o", g=2))

            # software-pipelined emission: batch b+1's projection halves are
            # interleaved into batch b's (ACT-gated) attention j-loop so PE
            # always has prioritized fill work; the heavier fill (half 1 +
            # V-transpose) lands before the large causal j-tiles
            emit_proj(0)
            emit_vtrans(0)
            for b in range(B):
                if b + 1 < B:
                    emit_proj(b + 1, halves=(0,))
                emit_attn(b, 0, 4)
                if b + 1 < B:
                    emit_proj(b + 1, halves=(1,))
                    emit_vtrans(b + 1)
                emit_attn(b, 4, N_QT)
                emit_oproj(b)

                # ---- output projection (partial over this core's channels) ----

    nc.compile()
    return nc


def _get_nc():
    if "nc" not in _CACHE:
        _CACHE["nc"] = _build_nc()
    return _CACHE["nc"]


def make_in_maps(x, Wq, bq, Wk, bk, Wv, bv, Wo):
    """Host-side sharding: returns per-core input dicts."""
    xt = np.ascontiguousarray(np.transpose(np.asarray(x, np.float32), (0, 2, 1)))
    mask = (np.arange(896, dtype=np.int64)[None, :]
            >= (np.arange(128, dtype=np.int64)[:, None] + 384)).astype(np.float32)
    ident = np.eye(128, dtype=np.float32)
    ones16 = np.ones((128, N_KC), dtype=np.float32)
    in_maps = []
    for i in range(NCORES):
        r = slice(i * C, (i + 1) * C)
        in_maps.append({
            "xt": xt,
            "wqt": np.ascontiguousarray(np.asarray(Wq, np.float32)[r, :].T),
            "wkt": np.ascontiguousarray(np.asarray(Wk, np.float32)[r, :].T),
            "wvt": np.ascontiguousarray(np.asarray(Wv, np.float32)[r, :].T),
            "wot": np.ascontiguousarray(np.asarray(Wo, np.float32)[:, r].T),
            "bq": np.asarray(bq, np.float32)[r].reshape(C, 1),
            "bk": np.asarray(bk, np.float32)[r].reshape(C, 1),
            "bv": np.asarray(bv, np.float32)[r].reshape(C, 1),
            "maskbuf": mask,
            "ident": ident,
            "ones16": ones16,
        })
    return in_maps


def run_cores(in_maps):
    nc = _get_nc()
    res = run_bass_kernel_spmd(nc, in_maps, core_ids=list(range(NCORES)))
    return [r["out"] for r in res.results]


def kernel(x, mask, Wq, bq, Wk, bk, Wv, bv, Wo, bo):
    in_maps = make_in_maps(x, Wq, bq, Wk, bk, Wv, bv, Wo)
    partials = run_cores(in_maps)
    out = partials[0]
    for p in partials[1:]:
        out = out + p
    return (out + np.asarray(bo, np.float32)[None, None, :]).astype(np.float32)



# revision 17
# speedup vs baseline: 1.2908x; 1.2908x over previous
"""Multi-head causal attention (B=4, S=2048, H=1024, NH=16) on 8 trn2 cores.

Head-sharded tensor parallelism: core i computes heads {2i, 2i+1}.  fp8
DoubleRow matmuls (2x128 contraction at 0.5 cycles/row) carry the heavy
GEMMs with hi/lo error compensation (e4m3 + e5m2, 3 passes) for the QKV
projections; Q/K score matmuls run fp8-DR on e4m3 stores; P/V/ctx stay
bf16 for accuracy.  Attention uses transposed scores S^T[k,q] (fp8-DR),
exp on ACT -> bf16 P, then a flipped P@V (stationary P^T chunk, moving
V^T chunk) that yields ctx^T[token, dim] so softmax normalization is a
per-partition scalar multiply.  ctx^T is transposed back via PE and the
output projection + partial output (bf16) is DMA'd out; the 8 partial
outputs are summed on the host (tensor-parallel all-reduce) with bo.
"""
import numpy as np
import ml_dtypes

import concourse.bacc as bacc
import concourse.tile as tile
from concourse import mybir
from concourse.bass_utils import run_bass_kernel_spmd

F32 = mybir.dt.float32
BF16 = mybir.dt.bfloat16
E4 = mybir.dt.float8e4
E5 = mybir.dt.float8e5
AF = mybir.ActivationFunctionType
DR = mybir.MatmulPerfMode.DoubleRow
MULT = mybir.AluOpType.mult
ADD = mybir.AluOpType.add

B, S, H, NH = 4, 2048, 1024, 16
HD = H // NH            # 64
NCORES = 8
HPC = NH // NCORES      # 2 heads per core
C = HPC * HD            # 128 channels per core
SCALE = 1.0 / np.sqrt(HD)
WS = 32.0               # weight pre-scale (keeps fp8 hi in normal range)

N_J = S // 256          # 8 q-tiles of 256 per head
QK_PASSES = 3           # fp8 compensation passes for Q/K projections
V_PASSES = 3

_CACHE = {}


def _build_nc():
    nc = bacc.Bacc(name="mha_fp8")
    xh_d = nc.dram_tensor("xh", [B, H, S], E4, kind="ExternalInput")
    xl_d = nc.dram_tensor("xl", [B, H, S], E5, kind="ExternalInput")
    wqh_d = nc.dram_tensor("wqh", [128, 1024], E4, kind="ExternalInput")
    wql_d = nc.dram_tensor("wql", [128, 1024], E5, kind="ExternalInput")
    wkh_d = nc.dram_tensor("wkh", [128, 1024], E4, kind="ExternalInput")
    wkl_d = nc.dram_tensor("wkl", [128, 1024], E5, kind="ExternalInput")
    wvh_d = nc.dram_tensor("wvh", [128, 1024], E4, kind="ExternalInput")
    wvl_d = nc.dram_tensor("wvl", [128, 1024], E5, kind="ExternalInput")
    wo_d = nc.dram_tensor("wo", [128, 1024], BF16, kind="ExternalInput")
    bq_d = nc.dram_tensor("bq", [128, 1], F32, kind="ExternalInput")
    bk_d = nc.dram_tensor("bk", [128, 1], F32, kind="ExternalInput")
    bv_d = nc.dram_tensor("bv", [128, 1], F32, kind="ExternalInput")
    tri_d = nc.dram_tensor("tri", [128, 128], BF16, kind="ExternalInput")
    on_d = nc.dram_tensor("ones2x16", [128, 32], BF16, kind="ExternalInput")
    id_d = nc.dram_tensor("ident", [128, 128], BF16, kind="ExternalInput")
    out_d = nc.dram_tensor("out", [B, S, H], BF16, kind="ExternalOutput")

    with tile.TileContext(nc) as tc:
        with (
            tc.tile_pool(name="const", bufs=1) as cp,
            tc.tile_pool(name="xs", bufs=4) as xp,
            tc.tile_pool(name="qk", bufs=2) as qp,
            tc.tile_pool(name="vn", bufs=2) as vp,
            tc.tile_pool(name="pts", bufs=12) as pp,
            tc.tile_pool(name="ctx", bufs=2) as ctp,
            tc.tile_pool(name="small", bufs=8) as sp,
            tc.tile_pool(name="osb", bufs=2) as op_,
            tc.tile_pool(name="stp", bufs=2, space="PSUM") as stp,
            tc.tile_pool(name="accp", bufs=1, space="PSUM") as accp,
            tc.tile_pool(name="mixp", bufs=2, space="PSUM") as mixp,
            tc.tile_pool(name="trp", bufs=1, space="PSUM") as trp,
        ):
            # ---- constants ----
            wqh_s = cp.tile([128, 1024], E4)
            wql_s = cp.tile([128, 1024], E5)
            wkh_s = cp.tile([128, 1024], E4)
            wkl_s = cp.tile([128, 1024], E5)
            wvh_s = cp.tile([128, 1024], E4)
            wvl_s = cp.tile([128, 1024], E5)
            wo_s = cp.tile([128, 1024], BF16)
            bq_s = cp.tile([128, 1], F32)
            bk_s = cp.tile([128, 1], F32)
            bv_s = cp.tile([128, 1], F32)
            tri_s = cp.tile([128, 128], BF16)
            on_s = cp.tile([128, 32], BF16)
            id_s = cp.tile([128, 128], BF16)
            for s, d in ((wqh_s, wqh_d), (wql_s, wql_d), (wkh_s, wkh_d),
                         (wkl_s, wkl_d), (wvh_s, wvh_d), (wvl_s, wvl_d),
                         (wo_s, wo_d), (bq_s, bq_d), (bk_s, bk_d),
                         (bv_s, bv_d), (tri_s, tri_d), (on_s, on_d),
                         (id_s, id_d)):
                nc.sync.dma_start(s[:], d.ap())

            # weight views [p, c2, i, out]
            wviews = {
                "qh": wqh_s.rearrange("p (c i o) -> p c i o", c=4, i=2),
                "ql": wql_s.rearrange("p (c i o) -> p c i o", c=4, i=2),
                "kh": wkh_s.rearrange("p (c i o) -> p c i o", c=4, i=2),
                "kl": wkl_s.rearrange("p (c i o) -> p c i o", c=4, i=2),
                "vh": wvh_s.rearrange("p (c i o) -> p c i o", c=4, i=2),
                "vl": wvl_s.rearrange("p (c i o) -> p c i o", c=4, i=2),
            }

            tiles = {}

            def emit_xload(b, half):
                xh_s = xp.tile([128, 8192], E4, tag="xh", name=f"xh{b}_{half}")
                xl_s = xp.tile([128, 8192], E5, tag="xl", name=f"xl{b}_{half}")
                hsl = slice(half * 1024, (half + 1) * 1024)
                for s, d in ((xh_s, xh_d), (xl_s, xl_d)):
                    nc.sync.dma_start(
                        s.rearrange("p (c t) -> p c t", c=8),
                        d.ap()[b, :, hsl].rearrange("(c p) t -> p c t", p=128))
                tiles[(b, half, "x")] = (
                    xh_s.rearrange("p (c i t) -> p c i t", c=4, i=2),
                    xl_s.rearrange("p (c i t) -> p c i t", c=4, i=2))

            def get_qkt(b):
                if (b, "qt") not in tiles:
                    tiles[(b, "qt")] = qp.tile([128, 2048], E4, tag="qt",
                                               name=f"qt{b}")
                    tiles[(b, "kt")] = qp.tile([128, 2048], E4, tag="kt",
                                               name=f"kt{b}")
                    tiles[(b, "vn")] = vp.tile([128, 2080], BF16, tag="vn",
                                               name=f"vn{b}")
                return tiles[(b, "qt")], tiles[(b, "kt")], tiles[(b, "vn")]

            def emit_qkproj(b, half, st):
                # Q,K projections for one 512-token subtile
                qt, kt, _ = get_qkt(b)
                xh_v, xl_v = tiles[(b, half, "x")]
                tsl = slice(st * 512, (st + 1) * 512)
                csl = slice(half * 1024 + st * 512, half * 1024 + (st + 1) * 512)
                for wh, wl, bias, dst in (("qh", "ql", bq_s, qt),
                                          ("kh", "kl", bk_s, kt)):
                    pmm = mixp.tile([128, 512], F32, tag="mix",
                                    name=f"pp{wh}{b}_{half}_{st}")
                    passes = [(wviews[wh], xh_v), (wviews[wl], xh_v)]
                    if QK_PASSES >= 3:
                        passes.append((wviews[wh], xl_v))
                    first = True
                    for w_v, x_v in passes:
                        for c2 in range(4):
                            nc.tensor.matmul(
                                pmm[:], w_v[:, c2], x_v[:, c2, :, tsl],
                                start=first, stop=(w_v is passes[-1][0]
                                                   and x_v is passes[-1][1]
                                                   and c2 == 3),
                                perf_mode=DR)
                            first = False
                    nc.vector.tensor_scalar(dst[:, csl], pmm[:], 1.0 / WS,
                                            bias[:], op0=MULT, op1=ADD)

            def emit_vproj(b, half, tc8):
                # V projection, flipped: psum [128 tok, 128 ch] for one
                # 128-token chunk; output (raw v * WS) into vn (both heads)
                _, _, vn = get_qkt(b)
                xh_v, xl_v = tiles[(b, half, "x")]
                c = half * 8 + tc8
                tsl = slice(tc8 * 128, (tc8 + 1) * 128)
                vm = mixp.tile([128, 512], F32, tag="mix", name=f"vp{b}_{c}")
                passes = [(wviews["vh"], xh_v), (wviews["vl"], xh_v)]
                if V_PASSES >= 3:
                    passes.append((wviews["vh"], xl_v))
                first = True
                for w_v, x_v in passes:
                    for c2 in range(4):
                        nc.tensor.matmul(
                            vm[:, 0:128], x_v[:, c2, :, tsl], w_v[:, c2],
                            start=first, stop=(w_v is passes[-1][0]
                                               and x_v is passes[-1][1]
                                               and c2 == 3),
                            perf_mode=DR)
                        first = False
                # both heads in one op: dst cols {h*1040 + c*65 + 0..63}
                dst = vn.rearrange("p (h c e) -> p h c e", h=2, e=65)
                nc.vector.tensor_scalar(dst[:, :, c, 0:64],
                                        vm[:, 0:128].rearrange(
                                            "p (h d) -> p h d", h=2),
                                        1.0 / WS, None, op0=MULT)

            def emit_ones(b):
                _, _, vn = get_qkt(b)
                dst = vn.rearrange("p (h c e) -> p h c e", h=2, e=65)
                nc.sync.dma_start(
                    dst[:, :, :, 64],
                    on_d.ap().rearrange("p (h c) -> p h c", h=2))

            def emit_remap(b):
                qt, kt, _ = get_qkt(b)
                qdr = qp.tile([64, 4096], E4, tag="qdr", name=f"qdr{b}")
                kdr = qp.tile([64, 4096], E4, tag="kdr", name=f"kdr{b}")
                tiles[(b, "qdr")], tiles[(b, "kdr")] = qdr, kdr
                for dr, src in ((qdr, qt), (kdr, kt)):
                    drv = dr.rearrange("p (i s) -> p i s", i=2)
                    for h in range(2):
                        for i in range(2):
                            nc.sync.dma_start(
                                drv[h * 32:(h + 1) * 32, i],
                                src[h * 64 + i * 32: h * 64 + i * 32 + 32, :])

            def group_chunks(j):
                """Chunk groups for one 256-q j-tile: list of (chunks, widths,
                positions, total_w). Chunk nkc-1 is 128 wide (q-local 128:256)."""
                nkc = 2 * (j + 1)
                groups = []
                for g0 in range(0, nkc, 4):
                    cs = list(range(g0, min(g0 + 4, nkc)))
                    pos, w = [], []
                    for c in cs:
                        pos.append((c - g0) * 256)
                        w.append(128 if c == nkc - 1 else 256)
                    groups.append((cs, w, pos, pos[-1] + w[-1]))
                return groups

            def emit_qk_scores(b, h, j, g):
                qdr, kdr = tiles[(b, "qdr")], tiles[(b, "kdr")]
                qv = qdr.rearrange("p (i s) -> p i s", i=2)
                kv = kdr.rearrange("p (i s) -> p i s", i=2)
                hsl = slice(h * 32, (h + 1) * 32)
                cs, ws, poss, tw = g
                st = stp.tile([128, 1024], F32, tag="st",
                              name=f"st{b}_{h}_{j}_{cs[0]}")
                nkc = 2 * (j + 1)
                for c, w, pos in zip(cs, ws, poss):
                    q0 = j * 256 + (128 if c == nkc - 1 else 0)
                    nc.tensor.matmul(
                        st[:, pos:pos + w],
                        kv[hsl, :, c * 128:(c + 1) * 128],
                        qv[hsl, :, q0:q0 + w],
                        start=True, stop=True, perf_mode=DR)
                return st

            def emit_exp(b, h, j, g, st):
                cs, ws, poss, tw = g
                pt = pp.tile([128, 1024], BF16, tag="pt",
                             name=f"pt{b}_{h}_{j}_{cs[0]}")
                nc.scalar.activation(pt[:, 0:tw], st[:, 0:tw], AF.Exp,
                                     scale=float(SCALE))
                return pt

            def emit_mask(b, h, j, pts_map, eng):
                # tri-mask the two diagonal blocks (in pt, post-exp)
                nkc = 2 * (j + 1)
                for c in (nkc - 2, nkc - 1):
                    g_idx = c // 4
                    pos = (c % 4) * 256
                    pt = pts_map[g_idx]
                    eng.tensor_mul(pt[:, pos:pos + 128], pt[:, pos:pos + 128],
                                   tri_s[:])

            def emit_pv(b, h, j, pts_map, ctxT):
                _, _, vn = get_qkt(b)
                nkc = 2 * (j + 1)
                if (b, h, "acc") not in tiles:
                    tiles[(b, h, "acc")] = accp.tile(
                        [128, 512], F32, tag="acc", name=f"acc{b}_{h}")
                acc = tiles[(b, h, "acc")]
                sl = (j % 2) * 256
                accv = acc.rearrange("p (s q e) -> p s q e", s=2, q=2)
                nc.vector.memset(accv[:, j % 2, :, 0:65], 0.0)
                for qb in range(2):
                    qc = 2 * j + qb
                    for c in range(qc + 1):
                        pt = pts_map[c // 4]
                        pos = (c % 4) * 256 + (qb * 128 if c < nkc - 1 else 0)
                        nc.tensor.matmul(
                            acc[:, sl + qb * 128: sl + qb * 128 + 65],
                            pt[:, pos:pos + 128],
                            vn[:, h * 1040 + c * 65: h * 1040 + (c + 1) * 65],
                            start=False, stop=(c == qc),
                            skip_group_check=True)
                # denominators -> reciprocal; normalize into ctxT (bf16)
                den = sp.tile([128, 2], F32, tag="den", name=f"dn{b}_{h}_{j}")
                nc.vector.reciprocal(den[:], accv[:, j % 2, :, 64])
                for qb in range(2):
                    qc = 2 * j + qb
                    nc.vector.tensor_scalar(
                        ctxT[:, qc * 128 + h * 64: qc * 128 + (h + 1) * 64],
                        acc[:, sl + qb * 128: sl + qb * 128 + 64],
                        den[:, qb:qb + 1], None, op0=MULT)

            def emit_ctxtrans(b, qc, ctxT, ctx, eng):
                tp = trp.tile([128, 128], BF16, tag="tp", name=f"tp{b}_{qc}")
                nc.tensor.transpose(tp[:], ctxT[:, qc * 128:(qc + 1) * 128],
                                    id_s[:])
                # copy + per-channel V bias (deferred from the projection)
                nc.vector.tensor_scalar(ctx[:, qc * 128:(qc + 1) * 128], tp[:],
                                        bv_s[:], None, op0=ADD)

            def emit_oproj(b, qp_, ctx):
                osb = op_.tile([128, 2048], BF16, tag="osb",
                               name=f"ob{b}_{qp_}")
                for sub in range(2):
                    qc = 2 * qp_ + sub
                    for half in range(2):
                        om = mixp.tile([128, 512], F32, tag="mix",
                                       name=f"om{b}_{qc}_{half}")
                        nc.tensor.matmul(om[:], ctx[:, qc * 128:(qc + 1) * 128],
                                         wo_s[:, half * 512:(half + 1) * 512],
                                         start=True, stop=True)
                        nc.vector.tensor_copy(
                            osb[:, sub * 1024 + half * 512:
                                sub * 1024 + (half + 1) * 512], om[:])
                nc.sync.dma_start(
                    out_d.ap()[b, qp_ * 256:(qp_ + 1) * 256, :]
                    .rearrange("(g q) o -> q g o", g=2),
                    osb.rearrange("p (g o) -> p g o", g=2))

            def emit_proj_unit(b, unit):
                # 24 projection units per batch: 8 qk (half, st) + 16 v
                if unit < 4:
                    emit_qkproj(b, unit // 2, unit % 2)
                elif unit < 8:
                    pass  # spare slot (qk units are 4)
                else:
                    u = unit - 8
                    emit_vproj(b, u // 8, u % 8)

            # ---------------- emission schedule ----------------
            # batch 0: Q/K projections + remap up front; V-projection
            # chunks are emitted just-in-time inside the attention loop
            for half in range(2):
                emit_xload(0, half)
            for half in range(2):
                for st in range(2):
                    emit_qkproj(0, half, st)
            emit_ones(0)
            emit_remap(0)
            v0_units = [(h_, t_) for h_ in range(2) for t_ in range(8)]
            emit_vproj(0, *v0_units.pop(0))
            emit_vproj(0, *v0_units.pop(0))

            def proj_unit(b, unit):
                # units for batch b: 4 qk (half, st) then 16 v (half, tc8)
                if unit < 4:
                    emit_qkproj(b, unit // 2, unit % 2)
                else:
                    u = unit - 4
                    emit_vproj(b, u // 8, u % 8)

            for b in range(B):
                ctxT = ctp.tile([128, 2048], BF16, tag="ctxT", name=f"cT{b}")
                ctx = ctp.tile([128, 2048], BF16, tag="ctx", name=f"cx{b}")
                tiles[(b, "ctx")] = ctx
                tiles[(b, "ctxT")] = ctxT
                slots = [(h, j) for h in range(2) for j in range(N_J)]
                next_units = list(range(20)) if b + 1 < B else []
                pv_backlog = []
                n_trans = 0
                n_oproj = 0
                if b + 1 < B:
                    emit_xload(b + 1, 0)
                    emit_xload(b + 1, 1)

                last = b == B - 1

                def pop_pv():
                    nonlocal n_trans, n_oproj
                    ph, pj, ppts = pv_backlog.pop(0)
                    emit_pv(b, ph, pj, ppts, ctxT)
                    if last and ph == 1:
                        for qc in (2 * pj, 2 * pj + 1):
                            emit_ctxtrans(b, qc, ctxT, ctx, None)
                            n_trans += 1
                        while n_oproj < n_trans // 2:
                            emit_oproj(b, n_oproj, ctx)
                            n_oproj += 1

                prev_trans = list(range(16)) if b > 0 else []
                prev_oproj = list(range(S // 256)) if b > 0 else []
                for si, (h, j) in enumerate(slots):
                    if si == 6 and b + 1 < B:
                        emit_ones(b + 1)
                        emit_remap(b + 1)
                    groups = group_chunks(j)
                    pts_map = {}
                    for gi, g in enumerate(groups):
                        st_t = emit_qk_scores(b, h, j, g)
                        pts_map[gi] = emit_exp(b, h, j, g, st_t)
                    emit_mask(b, h, j, pts_map, nc.gpsimd)
                    pv_backlog.append((h, j, pts_map))
                    if len(pv_backlog) > 1:
                        pop_pv()
                    for _ in range(2):
                        if prev_trans:
                            qc = prev_trans.pop(0)
                            emit_ctxtrans(b - 1, qc, tiles[(b - 1, "ctxT")],
                                          tiles[(b - 1, "ctx")], None)
                    if prev_oproj and si % 2 == 1:
                        emit_oproj(b - 1, prev_oproj.pop(0),
                                   tiles[(b - 1, "ctx")])
                    if b == 0 and v0_units:
                        emit_vproj(0, *v0_units.pop(0))
                        if si >= 2 and v0_units:
                            emit_vproj(0, *v0_units.pop(0))
                    if next_units:
                        proj_unit(b + 1, next_units.pop(0))
                        if si >= 8 and next_units:
                            proj_unit(b + 1, next_units.pop(0))
                while pv_backlog:
                    pop_pv()
                while next_units:
                    proj_unit(b + 1, next_units.pop(0))
                while prev_trans:
                    qc = prev_trans.pop(0)
                    emit_ctxtrans(b - 1, qc, tiles[(b - 1, "ctxT")],
                                  tiles[(b - 1, "ctx")], None)
                while prev_oproj:
                    emit_oproj(b - 1, prev_oproj.pop(0), tiles[(b - 1, "ctx")])
                if last:
                    while n_trans < 16:
                        emit_ctxtrans(b, n_trans, ctxT, ctx, None)
                        n_trans += 1
                    while n_oproj < S // 256:
                        emit_oproj(b, n_oproj, ctx)
                        n_oproj += 1
    nc.compile()
    return nc


def _get_nc():
    if "nc" not in _CACHE:
        _CACHE["nc"] = _build_nc()
    return _CACHE["nc"]


def _split8(a, scale=1.0):
    hi = (a * scale).astype(ml_dtypes.float8_e4m3)
    lo = (a * scale - hi.astype(np.float32)).astype(ml_dtypes.float8_e5m2)
    return hi, lo


def make_in_maps(x, Wq, bq, Wk, bk, Wv, bv, Wo):
    """Host-side sharding: returns per-core input dicts."""
    xt = np.ascontiguousarray(np.transpose(np.asarray(x, np.float32), (0, 2, 1)))
    xh, xl = _split8(xt)
    tri = np.triu(np.ones((128, 128), np.float32)).astype(ml_dtypes.bfloat16)
    ones = np.ones((128, 32), dtype=ml_dtypes.bfloat16)
    ident = np.eye(128, dtype=ml_dtypes.bfloat16)

    def wlayout(Wt):
        # Wt: [1024 in, 128 out] -> [128 p, 4 c2, 2 i, 128 out] -> [128, 1024]
        a = Wt.reshape(4, 2, 128, 128).transpose(2, 0, 1, 3)
        return np.ascontiguousarray(a.reshape(128, 1024))

    in_maps = []
    for i in range(NCORES):
        r = slice(i * C, (i + 1) * C)
        wqt = np.asarray(Wq, np.float32)[r, :].T  # [1024 in, 128 out]
        wkt = np.asarray(Wk, np.float32)[r, :].T
        wvt = np.asarray(Wv, np.float32)[r, :].T
        wqh_, wql_ = _split8(wlayout(wqt), WS)
        wkh_, wkl_ = _split8(wlayout(wkt), WS)
        wvh_, wvl_ = _split8(wlayout(wvt), WS)
        in_maps.append({
            "xh": xh, "xl": xl,
            "wqh": wqh_, "wql": wql_, "wkh": wkh_, "wkl": wkl_,
            "wvh": wvh_, "wvl": wvl_,
            "wo": np.ascontiguousarray(
                np.asarray(Wo, np.float32)[:, r].T).astype(ml_dtypes.bfloat16),
            "bq": np.asarray(bq, np.float32)[r].reshape(C, 1),
            "bk": np.asarray(bk, np.float32)[r].reshape(C, 1),
            "bv": np.asarray(bv, np.float32)[r].reshape(C, 1),
            "tri": tri, "ones2x16": ones, "ident": ident,
        })
    return in_maps


def run_cores(in_maps):
    nc = _get_nc()
    res = run_bass_kernel_spmd(nc, in_maps, core_ids=list(range(NCORES)))
    return [r["out"] for r in res.results]


def kernel(x, mask, Wq, bq, Wk, bk, Wv, bv, Wo, bo):
    in_maps = make_in_maps(x, Wq, bq, Wk, bk, Wv, bv, Wo)
    partials = run_cores(in_maps)
    out = np.zeros((B, S, H), np.float32)
    for p in partials:
        out += np.asarray(p).astype(np.float32)
    return (out + np.asarray(bo, np.float32)[None, None, :]).astype(np.float32)


# revision 25
# speedup vs baseline: 1.3447x; 1.0417x over previous
"""Multi-head causal attention (B=4, S=2048, H=1024, NH=16) on 8 trn2 cores.

Head-sharded tensor parallelism: core i computes heads {2i, 2i+1}.  fp8
DoubleRow matmuls (2x128 contraction at 0.5 cycles/row) carry the heavy
GEMMs with hi/lo error compensation (e4m3 + e5m2, 3 passes) for the QKV
projections; Q/K score matmuls run fp8-DR on e4m3 stores; P/V/ctx stay
bf16 for accuracy.  Attention uses transposed scores S^T[k,q] (fp8-DR),
exp on ACT -> bf16 P, then a flipped P@V (stationary P^T chunk, moving
V^T chunk) that yields ctx^T[token, dim] so softmax normalization is a
per-partition scalar multiply.  ctx^T is transposed back via PE and the
output projection + partial output (bf16) is DMA'd out; the 8 partial
outputs are summed on the host (tensor-parallel all-reduce) with bo.
"""
import numpy as np
import ml_dtypes

import concourse.bacc as bacc
import concourse.tile as tile
from concourse import mybir
from concourse.bass_utils import run_bass_kernel_spmd

F32 = mybir.dt.float32
BF16 = mybir.dt.bfloat16
E4 = mybir.dt.float8e4
E5 = mybir.dt.float8e5
AF = mybir.ActivationFunctionType
DR = mybir.MatmulPerfMode.DoubleRow
MULT = mybir.AluOpType.mult
ADD = mybir.AluOpType.add

B, S, H, NH = 4, 2048, 1024, 16
HD = H // NH            # 64
NCORES = 8
HPC = NH // NCORES      # 2 heads per core
C = HPC * HD            # 128 channels per core
SCALE = 1.0 / np.sqrt(HD)
WS = 32.0               # weight pre-scale (keeps fp8 hi in normal range)

N_J = S // 256          # 8 q-tiles of 256 per head
QK_PASSES = 2           # fp8 compensation passes for Q/K projections
V_PASSES = 3

_CACHE = {}


def _build_nc():
    nc = bacc.Bacc(name="mha_fp8")
    xh_d = nc.dram_tensor("xh", [B, H, S], E4, kind="ExternalInput")
    xl_d = nc.dram_tensor("xl", [B, H, S], E5, kind="ExternalInput")
    wqh_d = nc.dram_tensor("wqh", [128, 1024], E4, kind="ExternalInput")
    wql_d = nc.dram_tensor("wql", [128, 1024], E5, kind="ExternalInput")
    wkh_d = nc.dram_tensor("wkh", [128, 1024], E4, kind="ExternalInput")
    wkl_d = nc.dram_tensor("wkl", [128, 1024], E5, kind="ExternalInput")
    wvh_d = nc.dram_tensor("wvh", [128, 1024], E4, kind="ExternalInput")
    wvl_d = nc.dram_tensor("wvl", [128, 1024], E5, kind="ExternalInput")
    wo_d = nc.dram_tensor("wo", [128, 1024], BF16, kind="ExternalInput")
    bq_d = nc.dram_tensor("bq", [128, 1], F32, kind="ExternalInput")
    bk_d = nc.dram_tensor("bk", [128, 1], F32, kind="ExternalInput")
    bv_d = nc.dram_tensor("bv", [128, 1], F32, kind="ExternalInput")
    tri_d = nc.dram_tensor("tri", [128, 128], BF16, kind="ExternalInput")
    on_d = nc.dram_tensor("ones2x16", [128, 32], BF16, kind="ExternalInput")
    id_d = nc.dram_tensor("ident", [128, 128], BF16, kind="ExternalInput")
    out_d = nc.dram_tensor("out", [B, S, H], BF16, kind="ExternalOutput")

    with tile.TileContext(nc) as tc:
        with (
            tc.tile_pool(name="const", bufs=1) as cp,
            tc.tile_pool(name="xs", bufs=4) as xp,
            tc.tile_pool(name="qk", bufs=2) as qp,
            tc.tile_pool(name="vn", bufs=2) as vp,
            tc.tile_pool(name="pts", bufs=12) as pp,
            tc.tile_pool(name="ctx", bufs=2) as ctp,
            tc.tile_pool(name="small", bufs=8) as sp,
            tc.tile_pool(name="osb", bufs=3) as op_,
            tc.tile_pool(name="stp", bufs=2, space="PSUM") as stp,
            tc.tile_pool(name="accp", bufs=1, space="PSUM") as accp,
            tc.tile_pool(name="mixp", bufs=2, space="PSUM") as mixp,
            tc.tile_pool(name="trp", bufs=1, space="PSUM") as trp,
        ):
            # ---- constants ----
            wqh_s = cp.tile([128, 1024], E4)
            wql_s = cp.tile([128, 1024], E5)
            wkh_s = cp.tile([128, 1024], E4)
            wkl_s = cp.tile([128, 1024], E5)
            wvh_s = cp.tile([128, 1024], E4)
            wvl_s = cp.tile([128, 1024], E5)
            wo_s = cp.tile([128, 1024], BF16)
            bq_s = cp.tile([128, 1], F32)
            bk_s = cp.tile([128, 1], F32)
            bv_s = cp.tile([128, 1], F32)
            tri_s = cp.tile([128, 128], BF16)
            on_s = cp.tile([128, 32], BF16)
            id_s = cp.tile([128, 128], BF16)
            for s, d in ((wqh_s, wqh_d), (wql_s, wql_d), (wkh_s, wkh_d),
                         (wkl_s, wkl_d), (wvh_s, wvh_d), (wvl_s, wvl_d),
                         (wo_s, wo_d), (bq_s, bq_d), (bk_s, bk_d),
                         (bv_s, bv_d), (tri_s, tri_d), (on_s, on_d),
                         (id_s, id_d)):
                nc.sync.dma_start(s[:], d.ap())

            # weight views [p, c2, i, out]
            wviews = {
                "qh": wqh_s.rearrange("p (c i o) -> p c i o", c=4, i=2),
                "ql": wql_s.rearrange("p (c i o) -> p c i o", c=4, i=2),
                "kh": wkh_s.rearrange("p (c i o) -> p c i o", c=4, i=2),
                "kl": wkl_s.rearrange("p (c i o) -> p c i o", c=4, i=2),
                "vh": wvh_s.rearrange("p (c i o) -> p c i o", c=4, i=2),
                "vl": wvl_s.rearrange("p (c i o) -> p c i o", c=4, i=2),
            }

            tiles = {}

            def emit_xload(b, half):
                xh_s = xp.tile([128, 8192], E4, tag="xh", name=f"xh{b}_{half}")
                xl_s = xp.tile([128, 8192], E5, tag="xl", name=f"xl{b}_{half}")
                hsl = slice(half * 1024, (half + 1) * 1024)
                for s, d in ((xh_s, xh_d), (xl_s, xl_d)):
                    nc.sync.dma_start(
                        s.rearrange("p (c t) -> p c t", c=8),
                        d.ap()[b, :, hsl].rearrange("(c p) t -> p c t", p=128))
                tiles[(b, half, "x")] = (
                    xh_s.rearrange("p (c i t) -> p c i t", c=4, i=2),
                    xl_s.rearrange("p (c i t) -> p c i t", c=4, i=2))

            def get_qkt(b):
                if (b, "qt") not in tiles:
                    tiles[(b, "qt")] = qp.tile([128, 2048], E4, tag="qt",
                                               name=f"qt{b}")
                    tiles[(b, "kt")] = qp.tile([128, 2048], E4, tag="kt",
                                               name=f"kt{b}")
                    tiles[(b, "vn")] = vp.tile([128, 2080], BF16, tag="vn",
                                               name=f"vn{b}")
                return tiles[(b, "qt")], tiles[(b, "kt")], tiles[(b, "vn")]

            def emit_qkproj(b, half, st):
                # Q,K projections for one 512-token subtile
                qt, kt, _ = get_qkt(b)
                xh_v, xl_v = tiles[(b, half, "x")]
                tsl = slice(st * 512, (st + 1) * 512)
                csl = slice(half * 1024 + st * 512, half * 1024 + (st + 1) * 512)
                for wh, wl, bias, dst in (("qh", "ql", bq_s, qt),
                                          ("kh", "kl", bk_s, kt)):
                    pmm = mixp.tile([128, 512], F32, tag="mix",
                                    name=f"pp{wh}{b}_{half}_{st}")
                    passes = [(wviews[wh], xh_v), (wviews[wl], xh_v)]
                    if QK_PASSES >= 3:
                        passes.append((wviews[wh], xl_v))
                    first = True
                    for w_v, x_v in passes:
                        for c2 in range(4):
                            nc.tensor.matmul(
                                pmm[:], w_v[:, c2], x_v[:, c2, :, tsl],
                                start=first, stop=(w_v is passes[-1][0]
                                                   and x_v is passes[-1][1]
                                                   and c2 == 3),
                                perf_mode=DR)
                            first = False
                    nc.vector.tensor_scalar(dst[:, csl], pmm[:], 1.0 / WS,
                                            bias[:], op0=MULT, op1=ADD)

            def emit_vproj(b, half, tc8):
                # V projection, flipped: psum [128 tok, 128 ch] for one
                # 128-token chunk; output (raw v * WS) into vn (both heads)
                _, _, vn = get_qkt(b)
                xh_v, xl_v = tiles[(b, half, "x")]
                c = half * 8 + tc8
                tsl = slice(tc8 * 128, (tc8 + 1) * 128)
                vm = mixp.tile([128, 512], F32, tag="mix", name=f"vp{b}_{c}")
                passes = [(wviews["vh"], xh_v), (wviews["vl"], xh_v)]
                if V_PASSES >= 3:
                    passes.append((wviews["vh"], xl_v))
                first = True
                for w_v, x_v in passes:
                    for c2 in range(4):
                        nc.tensor.matmul(
                            vm[:, 0:128], x_v[:, c2, :, tsl], w_v[:, c2],
                            start=first, stop=(w_v is passes[-1][0]
                                               and x_v is passes[-1][1]
                                               and c2 == 3),
                            perf_mode=DR)
                        first = False
                # both heads in one op: dst cols {h*1040 + c*65 + 0..63}
                dst = vn.rearrange("p (h c e) -> p h c e", h=2, e=65)
                nc.vector.tensor_scalar(dst[:, :, c, 0:64],
                                        vm[:, 0:128].rearrange(
                                            "p (h d) -> p h d", h=2),
                                        1.0 / WS, None, op0=MULT)

            def emit_ones(b):
                _, _, vn = get_qkt(b)
                dst = vn.rearrange("p (h c e) -> p h c e", h=2, e=65)
                nc.sync.dma_start(
                    dst[:, :, :, 64],
                    on_d.ap().rearrange("p (h c) -> p h c", h=2))

            def emit_remap(b):
                qt, kt, _ = get_qkt(b)
                qdr = qp.tile([64, 4096], E4, tag="qdr", name=f"qdr{b}")
                kdr = qp.tile([64, 4096], E4, tag="kdr", name=f"kdr{b}")
                tiles[(b, "qdr")], tiles[(b, "kdr")] = qdr, kdr
                for dr, src in ((qdr, qt), (kdr, kt)):
                    drv = dr.rearrange("p (i s) -> p i s", i=2)
                    for h in range(2):
                        for i in range(2):
                            nc.sync.dma_start(
                                drv[h * 32:(h + 1) * 32, i],
                                src[h * 64 + i * 32: h * 64 + i * 32 + 32, :])

            def group_chunks(j):
                """Chunk groups for one 256-q j-tile: list of (chunks, widths,
                positions, total_w). Chunk nkc-1 is 128 wide (q-local 128:256)."""
                nkc = 2 * (j + 1)
                groups = []
                for g0 in range(0, nkc, 4):
                    cs = list(range(g0, min(g0 + 4, nkc)))
                    pos, w = [], []
                    for c in cs:
                        pos.append((c - g0) * 256)
                        w.append(128 if c == nkc - 1 else 256)
                    groups.append((cs, w, pos, pos[-1] + w[-1]))
                return groups

            def emit_qk_scores(b, h, j, g):
                qdr, kdr = tiles[(b, "qdr")], tiles[(b, "kdr")]
                qv = qdr.rearrange("p (i s) -> p i s", i=2)
                kv = kdr.rearrange("p (i s) -> p i s", i=2)
                hsl = slice(h * 32, (h + 1) * 32)
                cs, ws, poss, tw = g
                st = stp.tile([128, 1024], F32, tag="st",
                              name=f"st{b}_{h}_{j}_{cs[0]}")
                nkc = 2 * (j + 1)
                for c, w, pos in zip(cs, ws, poss):
                    q0 = j * 256 + (128 if c == nkc - 1 else 0)
                    nc.tensor.matmul(
                        st[:, pos:pos + w],
                        kv[hsl, :, c * 128:(c + 1) * 128],
                        qv[hsl, :, q0:q0 + w],
                        start=True, stop=True, perf_mode=DR)
                return st

            def emit_exp(b, h, j, g, st):
                cs, ws, poss, tw = g
                pt = pp.tile([128, 1024], BF16, tag="pt",
                             name=f"pt{b}_{h}_{j}_{cs[0]}")
                nc.scalar.activation(pt[:, 0:tw], st[:, 0:tw], AF.Exp,
                                     scale=float(SCALE))
                return pt

            def emit_mask(b, h, j, pts_map, eng):
                # tri-mask the two diagonal blocks (in pt, post-exp)
                nkc = 2 * (j + 1)
                for c in (nkc - 2, nkc - 1):
                    g_idx = c // 4
                    pos = (c % 4) * 256
                    pt = pts_map[g_idx]
                    eng.tensor_mul(pt[:, pos:pos + 128], pt[:, pos:pos + 128],
                                   tri_s[:])

            def emit_pv(b, h, j, pts_map, ctxT):
                _, _, vn = get_qkt(b)
                nkc = 2 * (j + 1)
                if (b, h, "acc") not in tiles:
                    tiles[(b, h, "acc")] = accp.tile(
                        [128, 512], F32, tag="acc", name=f"acc{b}_{h}")
                acc = tiles[(b, h, "acc")]
                sl = (j % 2) * 256
                accv = acc.rearrange("p (s q e) -> p s q e", s=2, q=2)
                nc.vector.memset(accv[:, j % 2, :, 0:65], 0.0)
                for qb in range(2):
                    qc = 2 * j + qb
                    for c in range(qc + 1):
                        pt = pts_map[c // 4]
                        pos = (c % 4) * 256 + (qb * 128 if c < nkc - 1 else 0)
                        nc.tensor.matmul(
                            acc[:, sl + qb * 128: sl + qb * 128 + 65],
                            pt[:, pos:pos + 128],
                            vn[:, h * 1040 + c * 65: h * 1040 + (c + 1) * 65],
                            start=False, stop=(c == qc),
                            skip_group_check=True)
                # denominators -> reciprocal; normalize into ctxT (bf16)
                den = sp.tile([128, 2], F32, tag="den", name=f"dn{b}_{h}_{j}")
                nc.vector.reciprocal(den[:], accv[:, j % 2, :, 64])
                for qb in range(2):
                    qc = 2 * j + qb
                    nc.vector.tensor_scalar(
                        ctxT[:, qc * 128 + h * 64: qc * 128 + (h + 1) * 64],
                        acc[:, sl + qb * 128: sl + qb * 128 + 64],
                        den[:, qb:qb + 1], None, op0=MULT)

            def emit_ctxtrans(b, qc, ctxT, ctx, eng):
                tp = trp.tile([128, 128], BF16, tag="tp", name=f"tp{b}_{qc}")
                nc.tensor.transpose(tp[:], ctxT[:, qc * 128:(qc + 1) * 128],
                                    id_s[:])
                # copy + per-channel V bias (deferred from the projection)
                nc.vector.tensor_scalar(ctx[:, qc * 128:(qc + 1) * 128], tp[:],
                                        bv_s[:], None, op0=ADD)

            def emit_oproj(b, qp_, ctx):
                osb = op_.tile([128, 2048], BF16, tag="osb",
                               name=f"ob{b}_{qp_}")
                for sub in range(2):
                    qc = 2 * qp_ + sub
                    for half in range(2):
                        om = mixp.tile([128, 512], F32, tag="mix",
                                       name=f"om{b}_{qc}_{half}")
                        nc.tensor.matmul(om[:], ctx[:, qc * 128:(qc + 1) * 128],
                                         wo_s[:, half * 512:(half + 1) * 512],
                                         start=True, stop=True)
                        nc.vector.tensor_copy(
                            osb[:, sub * 1024 + half * 512:
                                sub * 1024 + (half + 1) * 512], om[:])
                nc.sync.dma_start(
                    out_d.ap()[b, qp_ * 256:(qp_ + 1) * 256, :]
                    .rearrange("(g q) o -> q g o", g=2),
                    osb.rearrange("p (g o) -> p g o", g=2))

            def emit_proj_unit(b, unit):
                # 24 projection units per batch: 8 qk (half, st) + 16 v
                if unit < 4:
                    emit_qkproj(b, unit // 2, unit % 2)
                elif unit < 8:
                    pass  # spare slot (qk units are 4)
                else:
                    u = unit - 8
                    emit_vproj(b, u // 8, u % 8)

            # ---------------- emission schedule ----------------
            # batch 0: Q/K projections + remap up front; V-projection
            # chunks are emitted just-in-time inside the attention loop
            for half in range(2):
                emit_xload(0, half)
            for half in range(2):
                for st in range(2):
                    emit_qkproj(0, half, st)
            emit_ones(0)
            emit_remap(0)
            v0_units = [(h_, t_) for h_ in range(2) for t_ in range(8)]
            emit_vproj(0, *v0_units.pop(0))
            emit_vproj(0, *v0_units.pop(0))

            def proj_unit(b, unit):
                # units for batch b: 4 qk (half, st) then 16 v (half, tc8)
                if unit < 4:
                    emit_qkproj(b, unit // 2, unit % 2)
                else:
                    u = unit - 4
                    emit_vproj(b, u // 8, u % 8)

            for b in range(B):
                ctxT = ctp.tile([128, 2048], BF16, tag="ctxT", name=f"cT{b}")
                ctx = ctp.tile([128, 2048], BF16, tag="ctx", name=f"cx{b}")
                tiles[(b, "ctx")] = ctx
                tiles[(b, "ctxT")] = ctxT
                slots = [(h, j) for h in range(2) for j in range(N_J)]
                next_units = list(range(20)) if b + 1 < B else []
                pv_backlog = []
                n_trans = 0
                n_oproj = 0
                if b + 1 < B:
                    emit_xload(b + 1, 0)
                    emit_xload(b + 1, 1)

                last = b == B - 1

                def pop_pv():
                    nonlocal n_trans, n_oproj
                    ph, pj, ppts = pv_backlog.pop(0)
                    emit_pv(b, ph, pj, ppts, ctxT)
                    if last and ph == 1:
                        for qc in (2 * pj, 2 * pj + 1):
                            emit_ctxtrans(b, qc, ctxT, ctx, None)
                            n_trans += 1
                        while n_oproj < n_trans // 2:
                            emit_oproj(b, n_oproj, ctx)
                            n_oproj += 1

                prev_trans = list(range(16)) if b > 0 else []
                prev_oproj = list(range(S // 256)) if b > 0 else []
                for si, (h, j) in enumerate(slots):
                    if si == 6 and b + 1 < B:
                        emit_ones(b + 1)
                        emit_remap(b + 1)
                    groups = group_chunks(j)
                    pts_map = {}
                    for gi, g in enumerate(groups):
                        st_t = emit_qk_scores(b, h, j, g)
                        pts_map[gi] = emit_exp(b, h, j, g, st_t)
                    emit_mask(b, h, j, pts_map, nc.gpsimd)
                    pv_backlog.append((h, j, pts_map))
                    if len(pv_backlog) > 1:
                        pop_pv()
                    for _ in range(2):
                        if prev_trans:
                            qc = prev_trans.pop(0)
                            emit_ctxtrans(b - 1, qc, tiles[(b - 1, "ctxT")],
                                          tiles[(b - 1, "ctx")], None)
                    if prev_oproj and si % 2 == 1:
                        emit_oproj(b - 1, prev_oproj.pop(0),
                                   tiles[(b - 1, "ctx")])
                    if b == 0 and v0_units:
                        emit_vproj(0, *v0_units.pop(0))
                        if si >= 2 and v0_units:
                            emit_vproj(0, *v0_units.pop(0))
                    if next_units:
                        proj_unit(b + 1, next_units.pop(0))
                        if si >= 5 and next_units:
                            proj_unit(b + 1, next_units.pop(0))
                while pv_backlog:
                    pop_pv()
                while next_units:
                    proj_unit(b + 1, next_units.pop(0))
                while prev_trans:
                    qc = prev_trans.pop(0)
                    emit_ctxtrans(b - 1, qc, tiles[(b - 1, "ctxT")],
                                  tiles[(b - 1, "ctx")], None)
                while prev_oproj:
                    emit_oproj(b - 1, prev_oproj.pop(0), tiles[(b - 1, "ctx")])
                if last:
                    while n_trans < 16:
                        emit_ctxtrans(b, n_trans, ctxT, ctx, None)
                        n_trans += 1
                    while n_oproj < S // 256:
                        emit_oproj(b, n_oproj, ctx)
                        n_oproj += 1
    nc.compile()
    return nc


def _get_nc():
    if "nc" not in _CACHE:
        _CACHE["nc"] = _build_nc()
    return _CACHE["nc"]


def _split8(a, scale=1.0):
    hi = (a * scale).astype(ml_dtypes.float8_e4m3)
    lo = (a * scale - hi.astype(np.float32)).astype(ml_dtypes.float8_e5m2)
    return hi, lo


def make_in_maps(x, Wq, bq, Wk, bk, Wv, bv, Wo):
    """Host-side sharding: returns per-core input dicts."""
    xt = np.ascontiguousarray(np.transpose(np.asarray(x, np.float32), (0, 2, 1)))
    xh, xl = _split8(xt)
    tri = np.triu(np.ones((128, 128), np.float32)).astype(ml_dtypes.bfloat16)
    ones = np.ones((128, 32), dtype=ml_dtypes.bfloat16)
    ident = np.eye(128, dtype=ml_dtypes.bfloat16)

    def wlayout(Wt):
        # Wt: [1024 in, 128 out] -> [128 p, 4 c2, 2 i, 128 out] -> [128, 1024]
        a = Wt.reshape(4, 2, 128, 128).transpose(2, 0, 1, 3)
        return np.ascontiguousarray(a.reshape(128, 1024))

    in_maps = []
    for i in range(NCORES):
        r = slice(i * C, (i + 1) * C)
        wqt = np.asarray(Wq, np.float32)[r, :].T  # [1024 in, 128 out]
        wkt = np.asarray(Wk, np.float32)[r, :].T
        wvt = np.asarray(Wv, np.float32)[r, :].T
        wqh_, wql_ = _split8(wlayout(wqt), WS)
        wkh_, wkl_ = _split8(wlayout(wkt), WS)
        wvh_, wvl_ = _split8(wlayout(wvt), WS)
        in_maps.append({
            "xh": xh, "xl": xl,
            "wqh": wqh_, "wql": wql_, "wkh": wkh_, "wkl": wkl_,
            "wvh": wvh_, "wvl": wvl_,
            "wo": np.ascontiguousarray(
                np.asarray(Wo, np.float32)[:, r].T).astype(ml_dtypes.bfloat16),
            "bq": np.asarray(bq, np.float32)[r].reshape(C, 1),
            "bk": np.asarray(bk, np.float32)[r].reshape(C, 1),
            "bv": np.asarray(bv, np.float32)[r].reshape(C, 1),
            "tri": tri, "ones2x16": ones, "ident": ident,
        })
    return in_maps


def run_cores(in_maps):
    nc = _get_nc()
    res = run_bass_kernel_spmd(nc, in_maps, core_ids=list(range(NCORES)))
    return [r["out"] for r in res.results]


def kernel(x, mask, Wq, bq, Wk, bk, Wv, bv, Wo, bo):
    in_maps = make_in_maps(x, Wq, bq, Wk, bk, Wv, bv, Wo)
    partials = run_cores(in_maps)
    out = np.zeros((B, S, H), np.float32)
    for p in partials:
        out += np.asarray(p).astype(np.float32)
    return (out + np.asarray(bo, np.float32)[None, None, :]).astype(np.float32)


# revision 54
# speedup vs baseline: 1.4107x; 1.0491x over previous
"""Multi-head causal attention (B=4, S=2048, H=1024, NH=16) on 8 trn2 cores.

Head-sharded tensor parallelism: core i computes heads {2i, 2i+1}.  fp8
DoubleRow matmuls (2x128 contraction at 0.5 cycles/row) carry the heavy
GEMMs with hi/lo error compensation (e4m3 + e5m2, 3 passes) for the QKV
projections; Q/K score matmuls run fp8-DR on e4m3 stores; P/V/ctx stay
bf16 for accuracy.  Attention uses transposed scores S^T[k,q] (fp8-DR),
exp on ACT -> bf16 P, then a flipped P@V (stationary P^T chunk, moving
V^T chunk) that yields ctx^T[token, dim] so softmax normalization is a
per-partition scalar multiply.  ctx^T is transposed back via PE and the
output projection + partial output (bf16) is DMA'd out; the 8 partial
outputs are summed on the host (tensor-parallel all-reduce) with bo.
"""
import numpy as np
import ml_dtypes

import concourse.bacc as bacc
import concourse.tile as tile
from concourse import mybir
from concourse.bass_utils import run_bass_kernel_spmd

F32 = mybir.dt.float32
BF16 = mybir.dt.bfloat16
E4 = mybir.dt.float8e4
E5 = mybir.dt.float8e5
AF = mybir.ActivationFunctionType
DR = mybir.MatmulPerfMode.DoubleRow
MULT = mybir.AluOpType.mult
ADD = mybir.AluOpType.add

B, S, H, NH = 4, 2048, 1024, 16
HD = H // NH            # 64
NCORES = 8
HPC = NH // NCORES      # 2 heads per core
C = HPC * HD            # 128 channels per core
SCALE = 1.0 / np.sqrt(HD)
WS = 32.0               # weight pre-scale (keeps fp8 hi in normal range)

N_J = S // 256          # 8 q-tiles of 256 per head
QK_PASSES = 2           # fp8 compensation passes for Q/K projections
V_PASSES = 3

_CACHE = {}


def _build_nc():
    nc = bacc.Bacc(name="mha_fp8")
    xh_d = nc.dram_tensor("xh", [B, H, S], E4, kind="ExternalInput")
    xl_d = nc.dram_tensor("xl", [B, H, S], E5, kind="ExternalInput")
    wqh_d = nc.dram_tensor("wqh", [128, 1024], E4, kind="ExternalInput")
    wql_d = nc.dram_tensor("wql", [128, 1024], E5, kind="ExternalInput")
    wkh_d = nc.dram_tensor("wkh", [128, 1024], E4, kind="ExternalInput")
    wkl_d = nc.dram_tensor("wkl", [128, 1024], E5, kind="ExternalInput")
    wvh_d = nc.dram_tensor("wvh", [128, 1024], E4, kind="ExternalInput")
    wvl_d = nc.dram_tensor("wvl", [128, 1024], E5, kind="ExternalInput")
    wo_d = nc.dram_tensor("wo", [128, 1024], BF16, kind="ExternalInput")
    bq_d = nc.dram_tensor("bq", [128, 1], F32, kind="ExternalInput")
    bk_d = nc.dram_tensor("bk", [128, 1], F32, kind="ExternalInput")
    bv_d = nc.dram_tensor("bv", [128, 1], F32, kind="ExternalInput")
    tri_d = nc.dram_tensor("tri", [128, 128], BF16, kind="ExternalInput")
    on_d = nc.dram_tensor("ones2x16", [128, 32], BF16, kind="ExternalInput")
    id_d = nc.dram_tensor("ident", [128, 128], BF16, kind="ExternalInput")
    out_d = nc.dram_tensor("out", [B, S, H], BF16, kind="ExternalOutput")

    with tile.TileContext(nc) as tc:
        with (
            tc.tile_pool(name="const", bufs=1) as cp,
            tc.tile_pool(name="xs", bufs=4) as xp,
            tc.tile_pool(name="qk", bufs=3) as qp,
            tc.tile_pool(name="vn", bufs=3) as vp,
            tc.tile_pool(name="pts", bufs=14) as pp,
            tc.tile_pool(name="ctx", bufs=3) as ctp,
            tc.tile_pool(name="small", bufs=8) as sp,
            tc.tile_pool(name="osb", bufs=3) as op_,
            tc.tile_pool(name="stp", bufs=2, space="PSUM") as stp,
            tc.tile_pool(name="accp", bufs=1, space="PSUM") as accp,
            tc.tile_pool(name="mixp", bufs=2, space="PSUM") as mixp,
            tc.tile_pool(name="trp", bufs=1, space="PSUM") as trp,
        ):
            # ---- constants ----
            wqh_s = cp.tile([128, 1024], E4)
            wql_s = cp.tile([128, 1024], E5)
            wkh_s = cp.tile([128, 1024], E4)
            wkl_s = cp.tile([128, 1024], E5)
            wvh_s = cp.tile([128, 1024], E4)
            wvl_s = cp.tile([128, 1024], E5)
            wo_s = cp.tile([128, 1024], BF16)
            bq_s = cp.tile([128, 1], F32)
            bk_s = cp.tile([128, 1], F32)
            bv_s = cp.tile([128, 1], F32)
            tri_s = cp.tile([128, 128], BF16)
            on_s = cp.tile([128, 32], BF16)
            id_s = cp.tile([128, 128], BF16)
            for s, d in ((wqh_s, wqh_d), (wql_s, wql_d), (wkh_s, wkh_d),
                         (wkl_s, wkl_d), (wvh_s, wvh_d), (wvl_s, wvl_d),
                         (wo_s, wo_d), (bq_s, bq_d), (bk_s, bk_d),
                         (bv_s, bv_d), (tri_s, tri_d), (on_s, on_d),
                         (id_s, id_d)):
                nc.sync.dma_start(s[:], d.ap())

            # weight views [p, c2, i, out]
            wviews = {
                "qh": wqh_s.rearrange("p (c i o) -> p c i o", c=4, i=2),
                "ql": wql_s.rearrange("p (c i o) -> p c i o", c=4, i=2),
                "kh": wkh_s.rearrange("p (c i o) -> p c i o", c=4, i=2),
                "kl": wkl_s.rearrange("p (c i o) -> p c i o", c=4, i=2),
                "vh": wvh_s.rearrange("p (c i o) -> p c i o", c=4, i=2),
                "vl": wvl_s.rearrange("p (c i o) -> p c i o", c=4, i=2),
            }

            tiles = {}

            def emit_xload(b, half):
                xh_s = xp.tile([128, 8192], E4, tag="xh", name=f"xh{b}_{half}")
                xl_s = xp.tile([128, 8192], E5, tag="xl", name=f"xl{b}_{half}")
                hsl = slice(half * 1024, (half + 1) * 1024)
                for s, d in ((xh_s, xh_d), (xl_s, xl_d)):
                    nc.sync.dma_start(
                        s.rearrange("p (c t) -> p c t", c=8),
                        d.ap()[b, :, hsl].rearrange("(c p) t -> p c t", p=128))
                tiles[(b, half, "x")] = (
                    xh_s.rearrange("p (c i t) -> p c i t", c=4, i=2),
                    xl_s.rearrange("p (c i t) -> p c i t", c=4, i=2))

            def get_qkt(b):
                if (b, "qt") not in tiles:
                    tiles[(b, "qt")] = qp.tile([128, 2048], E4, tag="qt",
                                               name=f"qt{b}")
                    tiles[(b, "kt")] = qp.tile([128, 2048], E4, tag="kt",
                                               name=f"kt{b}")
                    tiles[(b, "vn")] = vp.tile([128, 2080], BF16, tag="vn",
                                               name=f"vn{b}")
                return tiles[(b, "qt")], tiles[(b, "kt")], tiles[(b, "vn")]

            def emit_qkproj(b, half, st):
                # Q,K projections for one 512-token subtile
                qt, kt, _ = get_qkt(b)
                xh_v, xl_v = tiles[(b, half, "x")]
                tsl = slice(st * 512, (st + 1) * 512)
                csl = slice(half * 1024 + st * 512, half * 1024 + (st + 1) * 512)
                for wh, wl, bias, dst in (("qh", "ql", bq_s, qt),
                                          ("kh", "kl", bk_s, kt)):
                    pmm = mixp.tile([128, 512], F32, tag="mix",
                                    name=f"pp{wh}{b}_{half}_{st}")
                    passes = [(wviews[wh], xh_v), (wviews[wl], xh_v)]
                    if QK_PASSES >= 3:
                        passes.append((wviews[wh], xl_v))
                    first = True
                    for w_v, x_v in passes:
                        for c2 in range(4):
                            nc.tensor.matmul(
                                pmm[:], w_v[:, c2], x_v[:, c2, :, tsl],
                                start=first, stop=(w_v is passes[-1][0]
                                                   and x_v is passes[-1][1]
                                                   and c2 == 3),
                                perf_mode=DR)
                            first = False
                    nc.vector.tensor_scalar(dst[:, csl], pmm[:], 1.0 / WS,
                                            bias[:], op0=MULT, op1=ADD)

            def emit_vproj(b, half, tc8):
                # V projection, flipped: psum [128 tok, 128 ch] for one
                # 128-token chunk; output (raw v * WS) into vn (both heads)
                _, _, vn = get_qkt(b)
                xh_v, xl_v = tiles[(b, half, "x")]
                c = half * 8 + tc8
                tsl = slice(tc8 * 128, (tc8 + 1) * 128)
                vm = mixp.tile([128, 512], F32, tag="mix", name=f"vp{b}_{c}")
                passes = [(wviews["vh"], xh_v), (wviews["vl"], xh_v)]
                if V_PASSES >= 3:
                    passes.append((wviews["vh"], xl_v))
                first = True
                for w_v, x_v in passes:
                    for c2 in range(4):
                        nc.tensor.matmul(
                            vm[:, 0:128], x_v[:, c2, :, tsl], w_v[:, c2],
                            start=first, stop=(w_v is passes[-1][0]
                                               and x_v is passes[-1][1]
                                               and c2 == 3),
                            perf_mode=DR)
                        first = False
                # both heads in one op: dst cols {h*1040 + c*65 + 0..63}
                dst = vn.rearrange("p (h c e) -> p h c e", h=2, e=65)
                nc.vector.tensor_scalar(dst[:, :, c, 0:64],
                                        vm[:, 0:128].rearrange(
                                            "p (h d) -> p h d", h=2),
                                        1.0 / WS, None, op0=MULT)

            def emit_ones(b):
                _, _, vn = get_qkt(b)
                dst = vn.rearrange("p (h c e) -> p h c e", h=2, e=65)
                nc.sync.dma_start(
                    dst[:, :, :, 64],
                    on_d.ap().rearrange("p (h c) -> p h c", h=2))

            def emit_remap(b):
                qt, kt, _ = get_qkt(b)
                qdr = qp.tile([64, 4096], E4, tag="qdr", name=f"qdr{b}")
                kdr = qp.tile([64, 4096], E4, tag="kdr", name=f"kdr{b}")
                tiles[(b, "qdr")], tiles[(b, "kdr")] = qdr, kdr
                for dr, src in ((qdr, qt), (kdr, kt)):
                    drv = dr.rearrange("p (i s) -> p i s", i=2)
                    for h in range(2):
                        for i in range(2):
                            nc.sync.dma_start(
                                drv[h * 32:(h + 1) * 32, i],
                                src[h * 64 + i * 32: h * 64 + i * 32 + 32, :])

            def group_chunks(j):
                """Chunk groups for one 256-q j-tile: list of (chunks, widths,
                positions, total_w). Chunk nkc-1 is 128 wide (q-local 128:256)."""
                nkc = 2 * (j + 1)
                groups = []
                for g0 in range(0, nkc, 4):
                    cs = list(range(g0, min(g0 + 4, nkc)))
                    pos, w = [], []
                    for c in cs:
                        pos.append((c - g0) * 256)
                        w.append(128 if c == nkc - 1 else 256)
                    groups.append((cs, w, pos, pos[-1] + w[-1]))
                return groups

            def emit_qk_scores(b, h, j, g):
                qdr, kdr = tiles[(b, "qdr")], tiles[(b, "kdr")]
                qv = qdr.rearrange("p (i s) -> p i s", i=2)
                kv = kdr.rearrange("p (i s) -> p i s", i=2)
                hsl = slice(h * 32, (h + 1) * 32)
                cs, ws, poss, tw = g
                st = stp.tile([128, 1024], F32, tag="st",
                              name=f"st{b}_{h}_{j}_{cs[0]}")
                nkc = 2 * (j + 1)
                for c, w, pos in zip(cs, ws, poss):
                    q0 = j * 256 + (128 if c == nkc - 1 else 0)
                    nc.tensor.matmul(
                        st[:, pos:pos + w],
                        kv[hsl, :, c * 128:(c + 1) * 128],
                        qv[hsl, :, q0:q0 + w],
                        start=True, stop=True, perf_mode=DR)
                return st

            def emit_exp(b, h, j, g, st):
                cs, ws, poss, tw = g
                pt = pp.tile([128, 1024], BF16, tag="pt",
                             name=f"pt{b}_{h}_{j}_{cs[0]}")
                nc.scalar.activation(pt[:, 0:tw], st[:, 0:tw], AF.Exp,
                                     scale=float(SCALE))
                return pt

            def emit_mask(b, h, j, pts_map, eng):
                # tri-mask the two diagonal blocks (in pt, post-exp)
                nkc = 2 * (j + 1)
                for c in (nkc - 2, nkc - 1):
                    g_idx = c // 4
                    pos = (c % 4) * 256
                    pt = pts_map[g_idx]
                    eng.tensor_mul(pt[:, pos:pos + 128], pt[:, pos:pos + 128],
                                   tri_s[:])

            def emit_pv(b, h, j, pts_map, ctxT):
                _, _, vn = get_qkt(b)
                nkc = 2 * (j + 1)
                if (b, h, "acc") not in tiles:
                    tiles[(b, h, "acc")] = accp.tile(
                        [128, 512], F32, tag="acc", name=f"acc{b}_{h}")
                acc = tiles[(b, h, "acc")]
                sl = (j % 2) * 256
                accv = acc.rearrange("p (s q e) -> p s q e", s=2, q=2)
                nc.vector.memset(accv[:, j % 2, :, 0:65], 0.0)
                for qb in range(2):
                    qc = 2 * j + qb
                    for c in range(qc + 1):
                        pt = pts_map[c // 4]
                        pos = (c % 4) * 256 + (qb * 128 if c < nkc - 1 else 0)
                        nc.tensor.matmul(
                            acc[:, sl + qb * 128: sl + qb * 128 + 65],
                            pt[:, pos:pos + 128],
                            vn[:, h * 1040 + c * 65: h * 1040 + (c + 1) * 65],
                            start=False, stop=(c == qc),
                            skip_group_check=True)
                # denominators -> reciprocal; normalize into ctxT (bf16)
                den = sp.tile([128, 2], F32, tag="den", name=f"dn{b}_{h}_{j}")
                nc.vector.reciprocal(den[:], accv[:, j % 2, :, 64])
                for qb in range(2):
                    qc = 2 * j + qb
                    nc.vector.tensor_scalar(
                        ctxT[:, qc * 128 + h * 64: qc * 128 + (h + 1) * 64],
                        acc[:, sl + qb * 128: sl + qb * 128 + 64],
                        den[:, qb:qb + 1], None, op0=MULT)

            def emit_ctxtrans(b, qc, ctxT, ctx, eng):
                tp = trp.tile([128, 128], BF16, tag="tp", name=f"tp{b}_{qc}")
                nc.tensor.transpose(tp[:], ctxT[:, qc * 128:(qc + 1) * 128],
                                    id_s[:])
                # copy + per-channel V bias (deferred from the projection)
                dst = ctx[:, qc * 128:(qc + 1) * 128]
                if b == B - 1 and qc % 2 == 0:
                    nc.scalar.add(dst, tp[:], bv_s[:])
                else:
                    nc.vector.tensor_scalar(dst, tp[:], bv_s[:], None, op0=ADD)

            def emit_oproj(b, qp_, ctx):
                osb = op_.tile([128, 2048], BF16, tag="osb",
                               name=f"ob{b}_{qp_}")
                for sub in range(2):
                    qc = 2 * qp_ + sub
                    for half in range(2):
                        om = mixp.tile([128, 512], F32, tag="mix",
                                       name=f"om{b}_{qc}_{half}")
                        nc.tensor.matmul(om[:], ctx[:, qc * 128:(qc + 1) * 128],
                                         wo_s[:, half * 512:(half + 1) * 512],
                                         start=True, stop=True)
                        dst = osb[:, sub * 1024 + half * 512:
                                  sub * 1024 + (half + 1) * 512]
                        if b == B - 1 and (qc + half) % 2 == 0:
                            nc.scalar.copy(dst, om[:])
                        else:
                            nc.vector.tensor_copy(dst, om[:])
                ov = out_d.ap()[b, qp_ * 256:(qp_ + 1) * 256, :] \
                    .rearrange("(g q) o -> q g o", g=2)
                sv2 = osb.rearrange("p (g o) -> p g o", g=2)
                for g in range(2):
                    nc.sync.dma_start(ov[:, g], sv2[:, g])

            def emit_proj_unit(b, unit):
                # 24 projection units per batch: 8 qk (half, st) + 16 v
                if unit < 4:
                    emit_qkproj(b, unit // 2, unit % 2)
                elif unit < 8:
                    pass  # spare slot (qk units are 4)
                else:
                    u = unit - 8
                    emit_vproj(b, u // 8, u % 8)

            # ---------------- emission schedule ----------------
            # batch 0: Q/K projections + remap up front; V-projection
            # chunks are emitted just-in-time inside the attention loop
            for half in range(2):
                emit_xload(0, half)
            for half in range(2):
                for st in range(2):
                    emit_qkproj(0, half, st)
            emit_ones(0)
            emit_remap(0)
            v0_units = [(h_, t_) for h_ in range(2) for t_ in range(8)]
            emit_vproj(0, *v0_units.pop(0))
            emit_vproj(0, *v0_units.pop(0))

            def proj_unit(b, unit):
                # units for batch b: 4 qk (half, st) then 16 v (half, tc8)
                if unit < 4:
                    emit_qkproj(b, unit // 2, unit % 2)
                else:
                    u = unit - 4
                    emit_vproj(b, u // 8, u % 8)

            for b in range(B):
                ctxT = ctp.tile([128, 2048], BF16, tag="ctxT", name=f"cT{b}")
                ctx = ctp.tile([128, 2048], BF16, tag="ctx", name=f"cx{b}")
                tiles[(b, "ctx")] = ctx
                tiles[(b, "ctxT")] = ctxT
                slots = [(h, j) for h in range(2) for j in range(N_J)]
                next_units = list(range(20)) if b + 1 < B else []
                pv_backlog = []
                n_trans = 0
                n_oproj = 0
                if b + 1 < B:
                    emit_xload(b + 1, 0)
                    emit_xload(b + 1, 1)

                last = b == B - 1

                def pop_pv():
                    nonlocal n_trans, n_oproj
                    ph, pj, ppts = pv_backlog.pop(0)
                    emit_pv(b, ph, pj, ppts, ctxT)
                    if last and ph == 1:
                        for qc in (2 * pj, 2 * pj + 1):
                            emit_ctxtrans(b, qc, ctxT, ctx, None)
                            n_trans += 1
                        while n_oproj < n_trans // 2:
                            emit_oproj(b, n_oproj, ctx)
                            n_oproj += 1

                prev_trans = list(range(16)) if b > 0 else []
                prev_oproj = list(range(S // 256)) if b > 0 else []
                for si, (h, j) in enumerate(slots):
                    if si == 6 and b + 1 < B:
                        emit_ones(b + 1)
                        emit_remap(b + 1)
                    groups = group_chunks(j)
                    pts_map = {}
                    for gi, g in enumerate(groups):
                        st_t = emit_qk_scores(b, h, j, g)
                        pts_map[gi] = emit_exp(b, h, j, g, st_t)
                    emit_mask(b, h, j, pts_map, nc.gpsimd)
                    pv_backlog.append((h, j, pts_map))
                    if len(pv_backlog) > 1:
                        pop_pv()
                    for _ in range(2):
                        if prev_trans:
                            qc = prev_trans.pop(0)
                            emit_ctxtrans(b - 1, qc, tiles[(b - 1, "ctxT")],
                                          tiles[(b - 1, "ctx")], None)
                    if prev_oproj and si % 2 == 1:
                        emit_oproj(b - 1, prev_oproj.pop(0),
                                   tiles[(b - 1, "ctx")])
                    if b == 0 and v0_units:
                        emit_vproj(0, *v0_units.pop(0))
                        if si >= 2 and v0_units:
                            emit_vproj(0, *v0_units.pop(0))
                    if next_units:
                        proj_unit(b + 1, next_units.pop(0))
                        if si >= 5 and next_units:
                            proj_unit(b + 1, next_units.pop(0))
                while pv_backlog:
                    pop_pv()
                while next_units:
                    proj_unit(b + 1, next_units.pop(0))
                while prev_trans:
                    qc = prev_trans.pop(0)
                    emit_ctxtrans(b - 1, qc, tiles[(b - 1, "ctxT")],
                                  tiles[(b - 1, "ctx")], None)
                while prev_oproj:
                    emit_oproj(b - 1, prev_oproj.pop(0), tiles[(b - 1, "ctx")])
                if last:
                    while n_trans < 16:
                        emit_ctxtrans(b, n_trans, ctxT, ctx, None)
                        n_trans += 1
                    while n_oproj < S // 256:
                        emit_oproj(b, n_oproj, ctx)
                        n_oproj += 1
    nc.compile()
    return nc


def _get_nc():
    if "nc" not in _CACHE:
        _CACHE["nc"] = _build_nc()
    return _CACHE["nc"]


def _split8(a, scale=1.0):
    hi = (a * scale).astype(ml_dtypes.float8_e4m3)
    lo = (a * scale - hi.astype(np.float32)).astype(ml_dtypes.float8_e5m2)
    return hi, lo


def make_in_maps(x, Wq, bq, Wk, bk, Wv, bv, Wo):
    """Host-side sharding: returns per-core input dicts."""
    xt = np.ascontiguousarray(np.transpose(np.asarray(x, np.float32), (0, 2, 1)))
    xh, xl = _split8(xt)
    tri = np.triu(np.ones((128, 128), np.float32)).astype(ml_dtypes.bfloat16)
    ones = np.ones((128, 32), dtype=ml_dtypes.bfloat16)
    ident = np.eye(128, dtype=ml_dtypes.bfloat16)

    def wlayout(Wt):
        # Wt: [1024 in, 128 out] -> [128 p, 4 c2, 2 i, 128 out] -> [128, 1024]
        a = Wt.reshape(4, 2, 128, 128).transpose(2, 0, 1, 3)
        return np.ascontiguousarray(a.reshape(128, 1024))

    in_maps = []
    for i in range(NCORES):
        r = slice(i * C, (i + 1) * C)
        wqt = np.asarray(Wq, np.float32)[r, :].T  # [1024 in, 128 out]
        wkt = np.asarray(Wk, np.float32)[r, :].T
        wvt = np.asarray(Wv, np.float32)[r, :].T
        wqh_, wql_ = _split8(wlayout(wqt), WS)
        wkh_, wkl_ = _split8(wlayout(wkt), WS)
        wvh_, wvl_ = _split8(wlayout(wvt), WS)
        in_maps.append({
            "xh": xh, "xl": xl,
            "wqh": wqh_, "wql": wql_, "wkh": wkh_, "wkl": wkl_,
            "wvh": wvh_, "wvl": wvl_,
            "wo": np.ascontiguousarray(
                np.asarray(Wo, np.float32)[:, r].T).astype(ml_dtypes.bfloat16),
            "bq": np.asarray(bq, np.float32)[r].reshape(C, 1),
            "bk": np.asarray(bk, np.float32)[r].reshape(C, 1),
            "bv": np.asarray(bv, np.float32)[r].reshape(C, 1),
            "tri": tri, "ones2x16": ones, "ident": ident,
        })
    return in_maps


def run_cores(in_maps):
    nc = _get_nc()
    res = run_bass_kernel_spmd(nc, in_maps, core_ids=list(range(NCORES)))
    return [r["out"] for r in res.results]


def kernel(x, mask, Wq, bq, Wk, bk, Wv, bv, Wo, bo):
    in_maps = make_in_maps(x, Wq, bq, Wk, bk, Wv, bv, Wo)
    partials = run_cores(in_maps)
    out = np.zeros((B, S, H), np.float32)
    for p in partials:
        out += np.asarray(p).astype(np.float32)
    return (out + np.asarray(bo, np.float32)[None, None, :]).astype(np.float32)
